# revision 24
# baseline (speedup 1.0000x reference)
"""Trainium2 Bass kernel for nn_ConstLoss_22746146800082 (fp8 Gram factorization).

loss = mean_{i != j} (Cq[i,j] - Ck[i,j])^2 with Cx the NxN pairwise cosine
matrix of feat_x (N=4096, D=1024).  With unit rows (eps terms cancel, the
NxN diagonal cancels exactly):

    loss*N*(N-1) = ||Gq||_F^2 + ||Gk||_F^2 - 2 ||Gx||_F^2

with DxD feature Grams Gq = Qn^T Qn, Gk = Kn^T Kn, Gx = Qn^T Kn - 2.7x
fewer MACs than the NxN route.

Block cover: the 1024 feature columns form 4 groups of 256; the needed
norms decompose into 36 inter-group blocks (10 Gq + 10 Gk + 16 Gx, using
||M|| = ||M^T|| to keep one orientation of each symmetric pair).  Each core
hosts FIVE generic block-slots over two host-packed column streams
S1 = [s1a|s1b], S2 = [s2a|s2b] (512 cols each, all 4096 rows, fp8):

    B0 = s1a^T s1a   B1 = s1a^T s1b   B2 = s2a^T s2b
    B3 = s1a^T s2a   B4 = s1b^T s2b

With q-core g: S1 = [qg|qg+1], S2 = [kg|kg+2] and k-core g: S1 = [kg|kg+1],
S2 = [qg+1|qg+3] (cores 0-3 / 4-7, group indices mod 4), the 40 slots cover
all 36 blocks and the combine weights are the SAME for every core:
total = sum_cores b0 + 2*b1 + b2 - 2*b3 - 2*b4.  40 slots vs 36 needed =
90% PE efficiency, 4MB DMA per core, 5 PSUM banks.

Each block accumulates [256 x 256] in one PSUM bank (two [128 x 256]
regions share the bank; start=True only on the bank's first matmul, the
second region initializes via the lazy 2KB zero-region).  All matmuls are
fp8e4 DoubleRow (K=256/pass, 0.5 cyc/row).  S1 streams first: B0/B1 finish
early and are squared out (fused square+row-sum) on ACT while S2 streams.
B3/B4 share their combine weight and live in one contiguous 2-bank PSUM
allocation, so the final tail is just one ACT square+accum over both and
one DVE copy+square of B2.  A few warmup matmuls into a scratch bank ramp
the PE to its full-speed p-state before the stream arrives.  No on-device
collectives; the host sums the per-bank partials with fixed weights.

Host prep: rows are L2-normalized in fp32, scaled by 16, quantized to
e4m3 (measured end-to-end rel err ~1.8e-3 vs fp64, gate is 2e-2).
"""

import numpy as np

import concourse.bass as bass
import concourse.mybir as mybir
import concourse.tile as tile
from concourse.vector_clock import ScopedClock
from concourse.bass_utils import run_bass_kernel_spmd

N_CORES = 8
N = 4096
D = 1024
P = 128

NCH = 16          # contraction chunks of 256 samples (2 DoubleRow k-tiles)
GW = 256          # feature-group width
W = 512           # stream width: two groups
SCALE = 16.0
NWARM = 8         # PE p-state warmup matmuls before the first stream chunk
PAD1 = 0          # gap-filler matmuls per chunk, S1 phase
PAD2 = 0          # gap-filler matmuls per chunk, S2 phase
PAD2_LAST = 12    # no S2 pads from this chunk on (PE already lags the tail)

F32 = mybir.dt.float32
FP8 = mybir.dt.float8e4
DR = mybir.MatmulPerfMode.DoubleRow
ACTF = mybir.ActivationFunctionType
ALU = mybir.AluOpType


class _TC(tile.TileContext):
    """TileContext whose kernel-tail drain splits its semaphore waits across
    preceding sync-engine NOPs: this container's walrus build rejects a Drain
    carrying more than one sync wait ("Too many sync wait commands")."""

    def _drain_and_barrier(self, tick_clock, wait_clock):
        nc = self.nc
        probe = nc.sync.nop(nofuse=True)
        wait_clock.add_sem_waits(
            probe.ins, ScopedClock({None: tick_clock.global_clock})
        )
        waits = list(probe.ins.sync_info.on_wait or []) if probe.ins.sync_info else []
        if probe.ins.sync_info is not None:
            probe.ins.sync_info.on_wait = waits[:1]
        for w in waits[1:]:
            n2 = nc.sync.nop(nofuse=True)
            n2.ins.sync_info = mybir.SyncInfo(on_wait=[w], on_update=[])
        nc.sync.drain()
        nc.all_engine_barrier()
        popped = nc._tile_sem_poison_stack.pop()
        assert popped is self._sem_poison
        nc.clear_and_free_semaphores(list(self.sems.allocated().values()))
        nc.all_engine_barrier()


MAX_WAITS_PER_INST = 1


def split_excess_waits(nc):
    """walrus (this build) rejects instructions carrying more than a couple
    of semaphore waits.  Hoist excess waits onto injected same-engine NOPs
    placed immediately before the offending instruction."""
    n = 0
    for f in nc.m.functions:
        for bb in f.blocks:
            insts = bb.instructions
            out = []
            changed = False
            for ins in insts:
                si = ins.sync_info
                waits = list(si.on_wait or []) if si is not None else []
                while len(waits) > MAX_WAITS_PER_INST:
                    take = waits[:MAX_WAITS_PER_INST]
                    waits = waits[MAX_WAITS_PER_INST:]
                    nop = mybir.InstNoOp(name=f"I-waitsplit-{n}", ins=[], outs=[])
                    n += 1
                    nop.engine = ins.engine
                    nop.sync_info = mybir.SyncInfo(on_wait=take, on_update=[])
                    out.append(nop)
                    changed = True
                if changed and si is not None:
                    si.on_wait = waits
                out.append(ins)
            if changed:
                bb.instructions = out
    return n


def build_program(sim_mode: bool = False):
    nc = bass.Bass(
        "TRN2", target_bir_lowering=False, debug=False, num_devices=N_CORES
    )
    # host-packed fp8 streams: [p, chunk, ktile, col]; sample = 256*chunk +
    # 128*ktile + p; cols 0:256 = group slot a, 256:512 = slot b.
    s1 = nc.dram_tensor("s1", [P, NCH, 2, W], FP8, kind="ExternalInput").ap()
    s2 = nc.dram_tensor("s2", [P, NCH, 2, W], FP8, kind="ExternalInput").ap()
    # per-partition partials: acc1 = banks 0,1 (early), acc2 = banks 2,3,4
    acc1_out = nc.dram_tensor("acc1", [P, 2], F32, kind="ExternalOutput").ap()
    acc2_out = nc.dram_tensor("acc2", [P, 2], F32, kind="ExternalOutput").ap()

    with _TC(nc) as tc:
        with (
            tc.tile_pool(name="stream", bufs=1) as stream,
            tc.tile_pool(name="fin", bufs=2) as fin,
            tc.tile_pool(name="psum", bufs=1, space="PSUM") as psum,
        ):
            t1 = stream.tile([P, NCH, 2, W], FP8, name="t1")
            t2 = stream.tile([P, NCH, 2, W], FP8, name="t2")
            # S1 first (banks 0,1 finish + square out early); last S2 chunks
            # ship in single-chunk DMAs so the final matmuls start early.
            split1 = [(0, 4), (4, 8), (8, 12), (12, 16)]
            split2 = [(0, 4), (4, 8), (8, 12), (12, 14), (14, 15), (15, 16)]
            for t_, src, split in ((t1, s1, split1), (t2, s2, split2)):
                for lo, hi in split:
                    cs = slice(lo, hi)
                    nc.sync.dma_start(out=t_[:, cs], in_=src[:, cs])

            banks = [
                psum.tile([P, 512], F32, name=f"bank{i}", tag=f"bank{i}")
                for i in range(3)
            ]
            # banks 3+4 share the combine weight (-2): one contiguous 2-bank
            # allocation so the tail squares them with a single ACT pass.
            banks34 = psum.tile([P, 1024], F32, name="banks34", tag="banks34")
            banks.append(banks34[:, 0:512])
            banks.append(banks34[:, 512:1024])
            scratch = psum.tile([P, 512], F32, name="scratch", tag="scratch")
            acc1 = fin.tile([P, 2], F32, name="acc1", bufs=1)
            acc2 = fin.tile([P, 2], F32, name="acc2", bufs=1)

            # warmup tile: memset once, then independent matmuls keep the PE
            # busy (and ramping to full p-state) until the stream arrives.
            wz = fin.tile([P, 2, GW], FP8, name="wz", bufs=1)
            nc.vector.memset(wz, 0.25)

            def pad_mm(src_tile, G):
                nc.tensor.matmul(
                    scratch[:, 256:512],
                    lhsT=src_tile[:, G, :, 0:P],
                    rhs=src_tile[:, G, :, 0:GW],
                    start=True,
                    stop=True,
                    perf_mode=DR,
                    skip_group_check=True,
                )

            def block_mm(bank, stat_t, stat_off, mov, first, last):
                for h in range(2):
                    lo = stat_off + h * P
                    nc.tensor.matmul(
                        bank[:, h * GW : (h + 1) * GW],
                        lhsT=stat_t[:, :, lo : lo + P],
                        rhs=mov,
                        start=first and h == 0,
                        stop=last,
                        perf_mode=DR,
                        skip_group_check=True,
                    )

            for i in range(NWARM):
                nc.tensor.matmul(
                    scratch[:, 0:GW],
                    lhsT=wz[:, :, 0:P],
                    rhs=wz,
                    start=True,
                    stop=True,
                    perf_mode=DR,
                    skip_group_check=True,
                )

            # ---- S1 phase: B0 = s1a^T s1a, B1 = s1a^T s1b --------------
            for G in range(NCH):
                fl = (G == 0, G == NCH - 1)
                c1 = t1[:, G]
                a1 = t1[:, G, :, 0:GW]
                b1 = t1[:, G, :, GW:W]
                block_mm(banks[0], c1, 0, a1, *fl)
                block_mm(banks[1], c1, 0, b1, *fl)
                for _ in range(PAD1):
                    pad_mm(t1, G)
            for i in range(2):
                sq = fin.tile([P, 512], F32, name=f"sq{i}", tag="sq")
                nc.scalar.activation(
                    sq, banks[i], ACTF.Square, accum_out=acc1[:, i : i + 1]
                )
            nc.sync.dma_start(out=acc1_out, in_=acc1)

            # ---- S2 phase: B2 = s2a^T s2b, B3 = s1a^T s2a, -------------
            # ----           B4 = s1b^T s2b                  -------------
            for G in range(NCH):
                fl = (G == 0, G == NCH - 1)
                a2 = t2[:, G, :, 0:GW]
                b2 = t2[:, G, :, GW:W]
                block_mm(banks[2], t2[:, G], 0, b2, *fl)
                block_mm(banks[3], t1[:, G], 0, a2, *fl)
                block_mm(banks[4], t1[:, G], GW, b2, *fl)
                if G < PAD2_LAST:
                    for _ in range(PAD2):
                        pad_mm(t2, G)

            # ---- tail: banks 3+4 in one ACT pass, bank 2 on DVE --------
            cx = fin.tile([P, 512], F32, name="cx", tag="cx")
            nc.vector.tensor_copy(cx, banks[2])
            sx = fin.tile([P, 512], F32, name="sx", tag="sx")
            nc.vector.scalar_tensor_tensor(
                sx, cx, 1.0, cx, op0=ALU.mult, op1=ALU.mult,
                accum_out=acc2[:, 0:1],
            )
            sq34 = fin.tile([P, 1024], F32, name="sq34", tag="sq34")
            nc.scalar.activation(
                sq34, banks34, ACTF.Square, accum_out=acc2[:, 1:2]
            )
            nc.sync.dma_start(out=acc2_out, in_=acc2)

    split_excess_waits(nc)
    return nc


_CACHE = {}


def _pack(m8, ga, gb):
    """[N, D] fp8 -> [P, NCH, 2, W] stream with groups (ga, gb) (mod 4)."""
    ga %= 4
    gb %= 4
    sub = np.concatenate(
        [m8[:, ga * GW : (ga + 1) * GW], m8[:, gb * GW : (gb + 1) * GW]], axis=1
    )
    return np.ascontiguousarray(sub.reshape(NCH, 2, P, W).transpose(2, 0, 1, 3))


def kernel(feat_q: np.ndarray, feat_k: np.ndarray) -> np.ndarray:
    import ml_dtypes

    fq = np.asarray(feat_q, dtype=np.float32)
    fk = np.asarray(feat_k, dtype=np.float32)
    assert fq.shape == (N, D) and fk.shape == (N, D)

    if "nc" not in _CACHE:
        _CACHE["nc"] = build_program()
    nc = _CACHE["nc"]

    e4 = ml_dtypes.float8_e4m3
    qn = fq / np.linalg.norm(fq, axis=1, keepdims=True)
    kn = fk / np.linalg.norm(fk, axis=1, keepdims=True)
    q8 = (qn * SCALE).astype(e4)
    k8 = (kn * SCALE).astype(e4)

    in_maps = []
    for g in range(4):  # q-cores
        in_maps.append({"s1": _pack(q8, g, g + 1), "s2": _pack(k8, g, g + 2)})
    for g in range(4):  # k-cores
        in_maps.append({"s1": _pack(k8, g, g + 1), "s2": _pack(q8, g + 1, g + 3)})
    res = run_bass_kernel_spmd(nc, in_maps, list(range(N_CORES)))

    # uniform weights: b0 + 2*b1 + b2 - 2*(b3 + b4)
    total = np.float64(0.0)
    for c in range(N_CORES):
        r = res.results[c]
        a1 = np.sum(r["acc1"].astype(np.float64), axis=0)
        a2 = np.sum(r["acc2"].astype(np.float64), axis=0)
        total += a1[0] + 2.0 * a1[1] + a2[0] - 2.0 * a2[1]
    loss = total / (np.float64(N) * (N - 1)) / np.float64(SCALE) ** 4
    return np.asarray(loss, dtype=np.float32)


if __name__ == "__main__":
    rng = np.random.default_rng(0)
    q = rng.standard_normal((N, D)).astype(np.float32)
    k = rng.standard_normal((N, D)).astype(np.float32)
    got = kernel(q, k)
    qn = q / np.linalg.norm(q, axis=1, keepdims=True)
    kn = k / np.linalg.norm(k, axis=1, keepdims=True)
    Gq = qn.T @ qn
    Gk = kn.T @ kn
    Gx = qn.T @ kn
    want = (np.sum(Gq * Gq) + np.sum(Gk * Gk) - 2 * np.sum(Gx * Gx)) / (
        N * (N - 1)
    )
    print("loss:", got, "want:", want, "rel:", abs(got - want) / abs(want))
